# revision 8
# baseline (speedup 1.0000x reference)
"""Causal self-attention TRN2 Bass kernel — iteration 2.

Problem: B=4, T=2048, C=1024, H=16 heads (HD=64), torch-Linear semantics
(y = x @ W.T + b), causal + padding mask, softmax, output projection.

Sharding: 8 cores = (batch b in 0..3) x (head-half in 0..1). Each core
handles one batch and 8 heads; the two half-cores of a batch produce
partial output projections that the host sums (plus bp).

Key scheduling idea vs iteration 1: the attention softmax (exp on the
ACT engine, ~1us per 128-k-tile) is the second-largest engine load after
the PE matmuls. Instead of running QKV projection, attention, and output
projection as serial phases (which leaves ACT idle during QKV/proj and
makes attention an ACT/PE lockstep), attention for q-chunk qc is
interleaved with the QKV projection chains of chunk qc+1 (attention only
needs K/V chunks <= qc), and the output projection of q-chunks 0..2 is
interleaved into attention of qc=3. The PE instruction stream then always
has dense matmul work while exp runs concurrently, and only qc=3's
projection remains as a short tail.

Numerics: QKV/proj matmuls in float32r; Q/K/V and softmax probs in bf16
(attention matmuls all-bf16; ~1.5e-3 worst-case rel err vs the 2e-2
harness gate). Normalization uses the exact DVE reciprocal + gpsimd row
broadcast (approx-recip / ACT ln-exp variants miscompute on this HW).
"""

from collections import deque

import ml_dtypes
import numpy as np

import concourse.mybir as mybir
import concourse.tile as tile
from concourse import bacc
from concourse.bass_utils import run_bass_kernel_spmd

F32 = mybir.dt.float32
F32R = mybir.dt.float32r
BF16 = mybir.dt.bfloat16
AF = mybir.ActivationFunctionType
ALU = mybir.AluOpType

B, T, C, H = 4, 2048, 1024, 16
HD = C // H          # 64
IC = C // 2          # 512 channels per core (8 heads)
NKT = T // 128       # 16 k-tiles
NCT = C // 128       # 8 contraction tiles for QKV
NEG = -1.0e30
SCALE = 1.0 / np.sqrt(HD)
D = 6                # flash pipeline depth (k-tiles between S and O)

_CACHE = {}


def _build():
    nc = bacc.Bacc("TRN2", target_bir_lowering=False, debug=False)

    xT_d = nc.dram_tensor("xT", [C, T], F32, kind="ExternalInput").ap()
    WqT_d = nc.dram_tensor("WqT", [C, IC], F32, kind="ExternalInput").ap()
    WkT_d = nc.dram_tensor("WkT", [C, IC], F32, kind="ExternalInput").ap()
    WvT_d = nc.dram_tensor("WvT", [C, IC], F32, kind="ExternalInput").ap()
    WpT_d = nc.dram_tensor("WpT", [IC, C], F32, kind="ExternalInput").ap()
    # packed constants: cols 0-3 bq*SCALE, 4-7 bk, 8-23 padding bias
    constf_d = nc.dram_tensor("constf", [128, 24], F32, kind="ExternalInput").ap()
    # packed bf16 constants: cols 0-127 identity, 128-255 causal mask bias
    constb_d = nc.dram_tensor("constb", [128, 256], BF16, kind="ExternalInput").ap()
    bv_d = nc.dram_tensor("bvr", [1, IC], F32, kind="ExternalInput").ap()
    ones1_d = nc.dram_tensor("ones1", [1, 128], F32, kind="ExternalInput").ap()
    ones8_d = nc.dram_tensor("ones8b", [128, 8], BF16, kind="ExternalInput").ap()
    out_d = nc.dram_tensor("out", [T, C], F32, kind="ExternalOutput").ap()

    with tile.TileContext(nc) as tc:
        with tc.tile_pool(name="pp", bufs=1) as pp:
            # Persistent SBUF state
            QT = pp.tile([128, 4 * T], BF16, name="QT")     # 4 head-pair tiles
            KT = pp.tile([128, 4 * T], BF16, name="KT")
            Vt = pp.tile([128, NKT * 520], BF16, name="Vt")  # [V|1] x 8 heads
            YT = pp.tile([128, 4 * T], F32R, name="YT")
            Wp_sb = pp.tile([128, 4 * C], F32R, name="Wp_sb")
            constf = pp.tile([128, 24], F32, name="constf")
            constb = pp.tile([128, 256], BF16, name="constb")
            bv_sb = pp.tile([1, IC], F32R, name="bv_sb")
            ones128 = pp.tile([1, 128], F32R, name="ones128")
            bq_sb = constf[:, 0:4]
            bk_sb = constf[:, 4:8]
            pad_sb = constf[:, 8:24]
            ident_sb = constb[:, 0:128]
            mask_sb = constb[:, 128:256]
            Vr = Vt.rearrange("p (k h c) -> p k h c", k=NKT, h=8, c=65)

            # PSUM pools stack: pss, pso stay for the whole kernel; ps1 (on
            # top) is swapped for the projection pool psp once QKV is done.
            pss = tc.alloc_tile_pool(name="pss", bufs=2, space="PSUM")
            pso = tc.alloc_tile_pool(name="pso", bufs=1, space="PSUM")
            ps1 = tc.alloc_tile_pool(name="ps1", bufs=2, space="PSUM")
            es = tc.alloc_tile_pool(name="es", bufs=D + 1)
            rp = tc.alloc_tile_pool(name="rp", bufs=1)
            p1 = tc.alloc_tile_pool(name="p1", bufs=1)
            xs = tc.alloc_tile_pool(name="xs", bufs=2)
            obp2 = tc.alloc_tile_pool(name="ob2", bufs=2)
            Wq_l = [p1.tile([128, 512], F32R, name=f"Wq{ct}") for ct in range(NCT)]
            Wk_l = [p1.tile([128, 512], F32R, name=f"Wk{ct}") for ct in range(NCT)]
            Wv_l = [p1.tile([128, 512], F32R, name=f"Wv{ct}") for ct in range(NCT)]

            def dma_x(ch):
                t0 = ch * 512
                xc = [xs.tile([128, 512], F32R, name=f"xc{ct}", tag=f"xc{ct}")
                      for ct in range(NCT)]
                for ct in range(NCT):
                    nc.sync.dma_start(
                        out=xc[ct][:],
                        in_=xT_d[ct * 128:(ct + 1) * 128, t0:t0 + 512].bitcast(F32R))
                return xc

            def q_chain(ch, g, xc):
                t0 = ch * 512
                pq = ps1.tile([128, 512], F32, name="pq", tag="p1ps")
                for ct in range(NCT):
                    nc.tensor.matmul(
                        out=pq[:], lhsT=Wq_l[ct][:, g * 128:(g + 1) * 128],
                        rhs=xc[ct][:], start=(ct == 0), stop=(ct == NCT - 1))
                nc.vector.tensor_scalar(
                    out=QT[:, g * T + t0: g * T + t0 + 512], in0=pq[:],
                    scalar1=SCALE, scalar2=bq_sb[:, g:g + 1],
                    op0=ALU.mult, op1=ALU.add)

            def k_chain(ch, g, xc):
                t0 = ch * 512
                pk = ps1.tile([128, 512], F32, name="pk", tag="p1ps")
                for ct in range(NCT):
                    nc.tensor.matmul(
                        out=pk[:], lhsT=Wk_l[ct][:, g * 128:(g + 1) * 128],
                        rhs=xc[ct][:], start=(ct == 0), stop=(ct == NCT - 1))
                nc.vector.tensor_scalar(
                    out=KT[:, g * T + t0: g * T + t0 + 512], in0=pk[:],
                    scalar1=bk_sb[:, g:g + 1], scalar2=None, op0=ALU.add)

            def v_chain(ch, ts, xc):
                kt = ch * 4 + ts
                pv_ = ps1.tile([128, 512], F32, name="pv_", tag="p1ps")
                for ct in range(NCT):
                    nc.tensor.matmul(
                        out=pv_[:], lhsT=xc[ct][:, ts * 128: ts * 128 + 128],
                        rhs=Wv_l[ct][:], start=(ct == 0), stop=False)
                nc.tensor.matmul(
                    out=pv_[:], lhsT=ones128[:], rhs=bv_sb[:],
                    start=False, stop=True)
                nc.vector.tensor_copy(Vr[:, kt, :, 0:64], pv_[:])

            def proj_tile(tt, oc, on_act, psp):
                po = psp.tile([128, 512], F32, name="po", tag="po")
                for g2 in range(4):
                    nc.tensor.matmul(
                        out=po[:],
                        lhsT=YT[:, g2 * T + tt * 128: g2 * T + tt * 128 + 128],
                        rhs=Wp_sb[:, g2 * C + oc * 512: g2 * C + oc * 512 + 512],
                        start=(g2 == 0), stop=(g2 == 3))
                ob = obp2.tile([128, 512], F32, name="ob", tag="ob")
                if on_act:
                    nc.scalar.copy(ob[:], po[:])
                else:
                    nc.vector.tensor_copy(ob[:], po[:])
                nc.sync.dma_start(
                    out=out_d[tt * 128:(tt + 1) * 128, oc * 512:(oc + 1) * 512],
                    in_=ob[:])

            def attn(qc, thunks):
                q0 = qc * 512
                kmax = 4 * qc + 4
                total_steps = 4 * (kmax + D)
                spacing = max(1, total_steps // max(1, len(thunks)))
                sidx = 0
                for g in range(4):
                    gq = g * T
                    oAB = pso.tile([65, 1024], F32, name="oAB", tag="o")
                    e_l = [None] * kmax
                    off_l = [None] * kmax
                    for step in range(kmax + D):
                        if step < kmax:
                            kt = step
                            k0 = kt * 128
                            toff = 128 * (kt - 4 * qc) if kt >= 4 * qc else 0
                            off_l[kt] = toff
                            diag = kt >= 4 * qc
                            sAB = pss.tile([128, 1024], F32, name="sAB", tag="sAB")
                            nc.tensor.matmul(
                                out=sAB[:, toff:512],
                                lhsT=KT[0:64, gq + k0: gq + k0 + 128],
                                rhs=QT[0:64, gq + q0 + toff: gq + q0 + 512],
                                start=True, stop=not diag)
                            nc.tensor.matmul(
                                out=sAB[:, 512 + toff:1024],
                                lhsT=KT[64:128, gq + k0: gq + k0 + 128],
                                rhs=QT[64:128, gq + q0 + toff: gq + q0 + 512],
                                start=True, stop=not diag, tile_position=(64, 0))
                            if diag:
                                # additive causal mask on the 128-wide
                                # diagonal band, via identity matmul
                                nc.tensor.matmul(
                                    out=sAB[:, toff:toff + 128],
                                    lhsT=ident_sb, rhs=mask_sb,
                                    start=False, stop=True)
                                nc.tensor.matmul(
                                    out=sAB[:, 512 + toff:512 + toff + 128],
                                    lhsT=ident_sb, rhs=mask_sb,
                                    start=False, stop=True)
                            eAB = es.tile([128, 1024], BF16, name="eAB", tag="eAB")
                            s3 = sAB.rearrange("p (h w) -> p h w", h=2, w=512)
                            e3 = eAB.rearrange("p (h w) -> p h w", h=2, w=512)
                            nc.scalar.activation(
                                e3[:, :, toff:512], s3[:, :, toff:512], AF.Exp,
                                bias=pad_sb[:, kt:kt + 1])
                            e_l[kt] = eAB
                        pv = step - D
                        if 0 <= pv < kmax:
                            toff = off_l[pv]
                            vbase = pv * 520
                            nc.tensor.matmul(
                                out=oAB[:, toff:512],
                                lhsT=Vt[:, vbase + 130 * g: vbase + 130 * g + 65],
                                rhs=e_l[pv][:, toff:512],
                                start=(pv == 0), stop=(pv == kmax - 1))
                            nc.tensor.matmul(
                                out=oAB[:, 512 + toff:1024],
                                lhsT=Vt[:, vbase + 130 * g + 65: vbase + 130 * g + 130],
                                rhs=e_l[pv][:, 512 + toff:1024],
                                start=(pv == 0), stop=(pv == kmax - 1))
                        sidx += 1
                        if thunks and sidx % spacing == 0:
                            thunks.popleft()()
                    # epilogue: normalize by rowsum (row 64), write Y^T.
                    # Exact DVE reciprocal (slow, ~6 cyc/elem, but the only
                    # normalization primitive that is bit-correct on this
                    # hardware runtime); the interleaved thunks keep the PE
                    # fed while this chain runs.
                    rA = rp.tile([1, 512], F32, name="rA", tag="rA")
                    rB = rp.tile([1, 512], F32, name="rB", tag="rB")
                    nc.vector.reciprocal(rA[:], oAB[64:65, 0:512])
                    nc.vector.reciprocal(rB[:], oAB[64:65, 512:1024])
                    rbA = rp.tile([64, 512], F32, name="rbA", tag="rbA")
                    rbB = rp.tile([64, 512], F32, name="rbB", tag="rbB")
                    nc.gpsimd.partition_broadcast(rbA[:], rA[:])
                    nc.gpsimd.partition_broadcast(rbB[:], rB[:])
                    nc.vector.tensor_mul(
                        YT[0:64, gq + q0: gq + q0 + 512],
                        oAB[0:64, 0:512], rbA[:])
                    nc.vector.tensor_mul(
                        YT[64:128, gq + q0: gq + q0 + 512],
                        oAB[0:64, 512:1024], rbB[:])
                while thunks:
                    thunks.popleft()()

            # ---------------- emission ----------------
            # chunk 0: critical-path DMA order, then QKV(0) inline
            xc0 = [xs.tile([128, 512], F32R, name=f"xc{ct}", tag=f"xc{ct}")
                   for ct in range(NCT)]
            for ct in range(NCT):
                cs = slice(ct * 128, (ct + 1) * 128)
                nc.sync.dma_start(out=xc0[ct][:], in_=xT_d[cs, 0:512].bitcast(F32R))
                nc.sync.dma_start(out=Wq_l[ct][:], in_=WqT_d[cs, :].bitcast(F32R))
            nc.sync.dma_start(out=constf[:], in_=constf_d)
            nc.sync.dma_start(out=constb[:], in_=constb_d)
            nc.sync.dma_start(out=bv_sb[:], in_=bv_d.bitcast(F32R))
            nc.sync.dma_start(out=ones128[:], in_=ones1_d.bitcast(F32R))
            for ct in range(NCT):
                cs = slice(ct * 128, (ct + 1) * 128)
                nc.sync.dma_start(out=Wk_l[ct][:], in_=WkT_d[cs, :].bitcast(F32R))
            for kt in range(NKT):
                nc.sync.dma_start(out=Vr[:, kt, :, 64], in_=ones8_d)
            for ct in range(NCT):
                cs = slice(ct * 128, (ct + 1) * 128)
                nc.sync.dma_start(out=Wv_l[ct][:], in_=WvT_d[cs, :].bitcast(F32R))
            for g in range(4):
                nc.sync.dma_start(
                    out=Wp_sb[:, g * C:(g + 1) * C],
                    in_=WpT_d[g * 128:(g + 1) * 128, :].bitcast(F32R))
            for g in range(4):
                q_chain(0, g, xc0)
            for g in range(4):
                k_chain(0, g, xc0)
            for ts in range(4):
                v_chain(0, ts, xc0)

            # attention(qc) interleaved with QKV(qc+1) chains
            psp = None
            for qc in range(4):
                thunks = deque()
                if qc < 3:
                    xc = dma_x(qc + 1)
                    for g in range(4):
                        thunks.append(lambda g=g, xc=xc: q_chain(qc + 1, g, xc))
                    for g in range(4):
                        thunks.append(lambda g=g, xc=xc: k_chain(qc + 1, g, xc))
                    for ts in range(4):
                        thunks.append(lambda ts=ts, xc=xc: v_chain(qc + 1, ts, xc))
                else:
                    # last QKV done: swap ps1's banks for the projection pool
                    # and interleave the projection of q-chunks 0..2
                    ps1.release()
                    psp = tc.alloc_tile_pool(name="psp", bufs=2, space="PSUM")
                    for tt in range(12):
                        for oc in range(2):
                            thunks.append(
                                lambda tt=tt, oc=oc: proj_tile(tt, oc, False, psp))
                attn(qc, thunks)

            # ---------------- projection tail (q-chunk 3) ----------------
            for tt in range(12, 16):
                for oc in range(2):
                    proj_tile(tt, oc, (tt + oc) % 2 == 1, psp)

            obp2.release()
            xs.release()
            p1.release()
            rp.release()
            es.release()
            psp.release()
            pso.release()
            pss.release()

    nc.compile()
    return nc


def _in_maps(x, Wk, bk, Wq, bq, Wv, bv, Wp, bp, padding_mask):
    maps = []
    mask_cols = np.arange(896)[None, :]
    mask_rows = np.arange(128)[:, None]
    maskneg = np.where(mask_rows <= mask_cols - 384, 0.0, NEG).astype(np.float32)
    identb = np.eye(128).astype(ml_dtypes.bfloat16)
    maskb = maskneg[:, 384:512].astype(ml_dtypes.bfloat16)
    constb = np.concatenate([identb, maskb], axis=1)
    for core in range(8):
        b, half = divmod(core, 2)
        hs = slice(half * IC, (half + 1) * IC)
        constf = np.concatenate([
            np.ascontiguousarray((bq[hs] * SCALE).reshape(4, 128).T),
            np.ascontiguousarray(bk[hs].reshape(4, 128).T),
            np.ascontiguousarray(
                np.where(padding_mask[b] != 0, 0.0, NEG)
                .astype(np.float32).reshape(NKT, 128).T),
        ], axis=1).astype(np.float32)
        maps.append({
            "xT": np.ascontiguousarray(x[b].T),
            "WqT": np.ascontiguousarray(Wq[hs, :].T),
            "WkT": np.ascontiguousarray(Wk[hs, :].T),
            "WvT": np.ascontiguousarray(Wv[hs, :].T),
            "WpT": np.ascontiguousarray(Wp[:, hs].T),
            "constf": np.ascontiguousarray(constf),
            "constb": np.ascontiguousarray(constb),
            "bvr": bv[hs].reshape(1, IC).copy(),
            "ones1": np.ones((1, 128), np.float32),
            "ones8b": np.ones((128, 8), ml_dtypes.bfloat16),
        })
    return maps


def _run(inputs, trace=False, **kw):
    if "nc" not in _CACHE:
        _CACHE["nc"] = _build()
    nc = _CACHE["nc"]
    ins = {k: np.asarray(v, dtype=np.float32) if k != "padding_mask"
           else np.asarray(v) for k, v in inputs.items()}
    maps = _in_maps(**ins)
    res = run_bass_kernel_spmd(nc, maps, core_ids=list(range(8)), trace=trace, **kw)
    bp = np.asarray(inputs["bp"], np.float32)
    y = np.empty((B, T, C), np.float32)
    for b in range(B):
        y[b] = res.results[2 * b]["out"] + res.results[2 * b + 1]["out"] + bp
    return y, res


def kernel(**inputs):
    y, _ = _run(inputs, trace=False)
    return y


# revision 10
# speedup vs baseline: 1.2473x; 1.2473x over previous
"""Causal self-attention TRN2 Bass kernel — iteration 2.

Problem: B=4, T=2048, C=1024, H=16 heads (HD=64), torch-Linear semantics
(y = x @ W.T + b), causal + padding mask, softmax, output projection.

Sharding: 8 cores = (batch b in 0..3) x (head-half in 0..1). Each core
handles one batch and 8 heads; the two half-cores of a batch produce
partial output projections that the host sums (plus bp).

Key scheduling idea vs iteration 1: the attention softmax (exp on the
ACT engine, ~1us per 128-k-tile) is the second-largest engine load after
the PE matmuls. Instead of running QKV projection, attention, and output
projection as serial phases (which leaves ACT idle during QKV/proj and
makes attention an ACT/PE lockstep), attention for q-chunk qc is
interleaved with the QKV projection chains of chunk qc+1 (attention only
needs K/V chunks <= qc), and the output projection of q-chunks 0..2 is
interleaved into attention of qc=3. The PE instruction stream then always
has dense matmul work while exp runs concurrently, and only qc=3's
projection remains as a short tail.

Numerics: QKV/proj matmuls in float32r; Q/K/V and softmax probs in bf16
(attention matmuls all-bf16; ~1.5e-3 worst-case rel err vs the 2e-2
harness gate). Normalization uses the exact DVE reciprocal + gpsimd row
broadcast (approx-recip / ACT ln-exp variants miscompute on this HW).
"""

from collections import deque

import ml_dtypes
import numpy as np

import concourse.mybir as mybir
import concourse.tile as tile
from concourse import bacc
from concourse.bass_utils import run_bass_kernel_spmd

F32 = mybir.dt.float32
F32R = mybir.dt.float32r
BF16 = mybir.dt.bfloat16
AF = mybir.ActivationFunctionType
ALU = mybir.AluOpType

B, T, C, H = 4, 2048, 1024, 16
HD = C // H          # 64
IC = C // 2          # 512 channels per core (8 heads)
NKT = T // 128       # 16 k-tiles
NCT = C // 128       # 8 contraction tiles for QKV
NEG = -1.0e30
SCALE = 1.0 / np.sqrt(HD)
D = 5                # flash pipeline depth (k-tiles between S and O)

_CACHE = {}


def _build():
    nc = bacc.Bacc("TRN2", target_bir_lowering=False, debug=False)

    xT_d = nc.dram_tensor("xT", [C, T], F32, kind="ExternalInput").ap()
    WqT_d = nc.dram_tensor("WqT", [C, IC], F32, kind="ExternalInput").ap()
    WkT_d = nc.dram_tensor("WkT", [C, IC], F32, kind="ExternalInput").ap()
    WvT_d = nc.dram_tensor("WvT", [C, IC], F32, kind="ExternalInput").ap()
    WpT_d = nc.dram_tensor("WpT", [IC, C], F32, kind="ExternalInput").ap()
    # packed constants: cols 0-3 bq*SCALE, 4-7 bk, 8-23 padding bias
    constf_d = nc.dram_tensor("constf", [128, 24], F32, kind="ExternalInput").ap()
    # packed bf16 constants: cols 0-127 identity, 128-255 causal mask bias
    constb_d = nc.dram_tensor("constb", [128, 256], BF16, kind="ExternalInput").ap()
    bv_d = nc.dram_tensor("bvr", [1, IC], F32, kind="ExternalInput").ap()
    ones1_d = nc.dram_tensor("ones1", [1, 128], F32, kind="ExternalInput").ap()
    ones8_d = nc.dram_tensor("ones8b", [128, 8], BF16, kind="ExternalInput").ap()
    out_d = nc.dram_tensor("out", [T, C], F32, kind="ExternalOutput").ap()

    with tile.TileContext(nc) as tc:
        with tc.tile_pool(name="pp", bufs=1) as pp:
            # Persistent SBUF state
            QT = pp.tile([128, 4 * T], BF16, name="QT")     # 4 head-pair tiles
            KT = pp.tile([128, 4 * T], BF16, name="KT")
            Vt = pp.tile([128, NKT * 520], BF16, name="Vt")  # [V|1] x 8 heads
            YT = pp.tile([128, 4 * T], F32R, name="YT")
            Wp_sb = pp.tile([128, 4 * C], F32R, name="Wp_sb")
            constf = pp.tile([128, 24], F32, name="constf")
            constb = pp.tile([128, 256], BF16, name="constb")
            bv_sb = pp.tile([1, IC], F32R, name="bv_sb")
            ones128 = pp.tile([1, 128], F32R, name="ones128")
            bq_sb = constf[:, 0:4]
            bk_sb = constf[:, 4:8]
            pad_sb = constf[:, 8:24]
            ident_sb = constb[:, 0:128]
            mask_sb = constb[:, 128:256]
            Vr = Vt.rearrange("p (k h c) -> p k h c", k=NKT, h=8, c=65)

            # PSUM pools stack: pss, pso stay for the whole kernel; ps1 (on
            # top) is swapped for the projection pool psp once QKV is done.
            pss = tc.alloc_tile_pool(name="pss", bufs=2, space="PSUM")
            pso = tc.alloc_tile_pool(name="pso", bufs=1, space="PSUM")
            ps1 = tc.alloc_tile_pool(name="ps1", bufs=2, space="PSUM")
            es = tc.alloc_tile_pool(name="es", bufs=D + 1)
            rp = tc.alloc_tile_pool(name="rp", bufs=1)
            p1 = tc.alloc_tile_pool(name="p1", bufs=1)
            xs = tc.alloc_tile_pool(name="xs", bufs=2)
            obp2 = tc.alloc_tile_pool(name="ob2", bufs=2)
            Wq_l = [p1.tile([128, 512], F32R, name=f"Wq{ct}") for ct in range(NCT)]
            Wk_l = [p1.tile([128, 512], F32R, name=f"Wk{ct}") for ct in range(NCT)]
            Wv_l = [p1.tile([128, 512], F32R, name=f"Wv{ct}") for ct in range(NCT)]

            def dma_x(ch):
                t0 = ch * 512
                xc = [xs.tile([128, 512], F32R, name=f"xc{ct}", tag=f"xc{ct}")
                      for ct in range(NCT)]
                for ct in range(NCT):
                    nc.sync.dma_start(
                        out=xc[ct][:],
                        in_=xT_d[ct * 128:(ct + 1) * 128, t0:t0 + 512].bitcast(F32R))
                return xc

            def q_chain(ch, g, xc):
                t0 = ch * 512
                pq = ps1.tile([128, 512], F32, name="pq", tag="p1ps")
                for ct in range(NCT):
                    nc.tensor.matmul(
                        out=pq[:], lhsT=Wq_l[ct][:, g * 128:(g + 1) * 128],
                        rhs=xc[ct][:], start=(ct == 0), stop=(ct == NCT - 1))
                nc.scalar.activation(
                    QT[:, g * T + t0: g * T + t0 + 512], pq[:], AF.Identity,
                    scale=SCALE, bias=bq_sb[:, g:g + 1])

            def k_chain(ch, g, xc):
                t0 = ch * 512
                pk = ps1.tile([128, 512], F32, name="pk", tag="p1ps")
                for ct in range(NCT):
                    nc.tensor.matmul(
                        out=pk[:], lhsT=Wk_l[ct][:, g * 128:(g + 1) * 128],
                        rhs=xc[ct][:], start=(ct == 0), stop=(ct == NCT - 1))
                nc.scalar.activation(
                    KT[:, g * T + t0: g * T + t0 + 512], pk[:], AF.Identity,
                    bias=bk_sb[:, g:g + 1])

            def v_chain(ch, ts, xc):
                kt = ch * 4 + ts
                pv_ = ps1.tile([128, 512], F32, name="pv_", tag="p1ps")
                for ct in range(NCT):
                    nc.tensor.matmul(
                        out=pv_[:], lhsT=xc[ct][:, ts * 128: ts * 128 + 128],
                        rhs=Wv_l[ct][:], start=(ct == 0), stop=False)
                nc.tensor.matmul(
                    out=pv_[:], lhsT=ones128[:], rhs=bv_sb[:],
                    start=False, stop=True)
                nc.scalar.copy(Vr[:, kt, :, 0:64], pv_[:])

            def proj_tile(tt, oc, on_act, psp):
                po = psp.tile([128, 512], F32, name="po", tag="po")
                for g2 in range(4):
                    nc.tensor.matmul(
                        out=po[:],
                        lhsT=YT[:, g2 * T + tt * 128: g2 * T + tt * 128 + 128],
                        rhs=Wp_sb[:, g2 * C + oc * 512: g2 * C + oc * 512 + 512],
                        start=(g2 == 0), stop=(g2 == 3))
                ob = obp2.tile([128, 512], F32, name="ob", tag="ob")
                if on_act:
                    nc.scalar.copy(ob[:], po[:])
                else:
                    nc.vector.tensor_copy(ob[:], po[:])
                nc.sync.dma_start(
                    out=out_d[tt * 128:(tt + 1) * 128, oc * 512:(oc + 1) * 512],
                    in_=ob[:])

            def attn(qc, thunks):
                q0 = qc * 512
                kmax = 4 * qc + 4
                total_steps = 4 * (kmax + D)
                spacing = max(1, total_steps // max(1, len(thunks)))
                sidx = 0
                for g in range(4):
                    gq = g * T
                    oAB = pso.tile([65, 1024], F32, name="oAB", tag="o")
                    e_l = [None] * kmax
                    off_l = [None] * kmax
                    for step in range(kmax + D):
                        if step < kmax:
                            kt = step
                            k0 = kt * 128
                            toff = 128 * (kt - 4 * qc) if kt >= 4 * qc else 0
                            off_l[kt] = toff
                            diag = kt >= 4 * qc
                            sAB = pss.tile([128, 1024], F32, name="sAB", tag="sAB")
                            nc.tensor.matmul(
                                out=sAB[:, toff:512],
                                lhsT=KT[0:64, gq + k0: gq + k0 + 128],
                                rhs=QT[0:64, gq + q0 + toff: gq + q0 + 512],
                                start=True, stop=not diag)
                            nc.tensor.matmul(
                                out=sAB[:, 512 + toff:1024],
                                lhsT=KT[64:128, gq + k0: gq + k0 + 128],
                                rhs=QT[64:128, gq + q0 + toff: gq + q0 + 512],
                                start=True, stop=not diag, tile_position=(64, 0))
                            if diag:
                                # additive causal mask on the 128-wide
                                # diagonal band, via identity matmul
                                nc.tensor.matmul(
                                    out=sAB[:, toff:toff + 128],
                                    lhsT=ident_sb, rhs=mask_sb,
                                    start=False, stop=True)
                                nc.tensor.matmul(
                                    out=sAB[:, 512 + toff:512 + toff + 128],
                                    lhsT=ident_sb, rhs=mask_sb,
                                    start=False, stop=True)
                            eAB = es.tile([128, 1024], BF16, name="eAB", tag="eAB")
                            s3 = sAB.rearrange("p (h w) -> p h w", h=2, w=512)
                            e3 = eAB.rearrange("p (h w) -> p h w", h=2, w=512)
                            nc.scalar.activation(
                                e3[:, :, toff:512], s3[:, :, toff:512], AF.Exp,
                                bias=pad_sb[:, kt:kt + 1])
                            e_l[kt] = eAB
                        pv = step - D
                        if 0 <= pv < kmax:
                            toff = off_l[pv]
                            vbase = pv * 520
                            nc.tensor.matmul(
                                out=oAB[:, toff:512],
                                lhsT=Vt[:, vbase + 130 * g: vbase + 130 * g + 65],
                                rhs=e_l[pv][:, toff:512],
                                start=(pv == 0), stop=(pv == kmax - 1))
                            nc.tensor.matmul(
                                out=oAB[:, 512 + toff:1024],
                                lhsT=Vt[:, vbase + 130 * g + 65: vbase + 130 * g + 130],
                                rhs=e_l[pv][:, 512 + toff:1024],
                                start=(pv == 0), stop=(pv == kmax - 1))
                        sidx += 1
                        if thunks and sidx % spacing == 0:
                            thunks.popleft()()
                    # epilogue: normalize by rowsum (row 64), write Y^T.
                    # Exact DVE reciprocal (slow, ~6 cyc/elem, but the only
                    # normalization primitive that is bit-correct on this
                    # hardware runtime); the interleaved thunks keep the PE
                    # fed while this chain runs.
                    ocp = rp.tile([65, 1024], F32, name="ocp", tag="ocp")
                    nc.scalar.copy(ocp[:], oAB[:])
                    rA = rp.tile([1, 512], F32, name="rA", tag="rA")
                    rB = rp.tile([1, 512], F32, name="rB", tag="rB")
                    nc.vector.reciprocal(rA[:], ocp[64:65, 0:512])
                    nc.vector.reciprocal(rB[:], ocp[64:65, 512:1024])
                    rbA = rp.tile([64, 512], F32, name="rbA", tag="rbA")
                    rbB = rp.tile([64, 512], F32, name="rbB", tag="rbB")
                    nc.gpsimd.partition_broadcast(rbA[:], rA[:])
                    nc.gpsimd.partition_broadcast(rbB[:], rB[:])
                    nc.vector.tensor_mul(
                        YT[0:64, gq + q0: gq + q0 + 512],
                        ocp[0:64, 0:512], rbA[:])
                    nc.vector.tensor_mul(
                        YT[64:128, gq + q0: gq + q0 + 512],
                        ocp[0:64, 512:1024], rbB[:])
                while thunks:
                    thunks.popleft()()

            # ---------------- emission ----------------
            # chunk 0: critical-path DMA order, then QKV(0) inline
            xc0 = [xs.tile([128, 512], F32R, name=f"xc{ct}", tag=f"xc{ct}")
                   for ct in range(NCT)]
            for ct in range(NCT):
                cs = slice(ct * 128, (ct + 1) * 128)
                nc.sync.dma_start(out=xc0[ct][:], in_=xT_d[cs, 0:512].bitcast(F32R))
                nc.sync.dma_start(out=Wq_l[ct][:], in_=WqT_d[cs, :].bitcast(F32R))
            nc.sync.dma_start(out=constf[:], in_=constf_d)
            nc.sync.dma_start(out=constb[:], in_=constb_d)
            nc.sync.dma_start(out=bv_sb[:], in_=bv_d.bitcast(F32R))
            nc.sync.dma_start(out=ones128[:], in_=ones1_d.bitcast(F32R))
            for ct in range(NCT):
                cs = slice(ct * 128, (ct + 1) * 128)
                nc.sync.dma_start(out=Wk_l[ct][:], in_=WkT_d[cs, :].bitcast(F32R))
            for kt in range(NKT):
                nc.sync.dma_start(out=Vr[:, kt, :, 64], in_=ones8_d)
            for ct in range(NCT):
                cs = slice(ct * 128, (ct + 1) * 128)
                nc.sync.dma_start(out=Wv_l[ct][:], in_=WvT_d[cs, :].bitcast(F32R))
            for g in range(4):
                nc.sync.dma_start(
                    out=Wp_sb[:, g * C:(g + 1) * C],
                    in_=WpT_d[g * 128:(g + 1) * 128, :].bitcast(F32R))
            for g in range(4):
                q_chain(0, g, xc0)
            for g in range(4):
                k_chain(0, g, xc0)
            for ts in range(4):
                v_chain(0, ts, xc0)

            # attention(qc) interleaved with QKV(qc+1) chains
            psp = None
            for qc in range(4):
                thunks = deque()
                if qc < 3:
                    xc = dma_x(qc + 1)
                    for g in range(4):
                        thunks.append(lambda g=g, xc=xc: q_chain(qc + 1, g, xc))
                    for g in range(4):
                        thunks.append(lambda g=g, xc=xc: k_chain(qc + 1, g, xc))
                    for ts in range(4):
                        thunks.append(lambda ts=ts, xc=xc: v_chain(qc + 1, ts, xc))
                else:
                    # last QKV done: swap ps1's banks for the projection pool
                    # and interleave the projection of q-chunks 0..2
                    ps1.release()
                    psp = tc.alloc_tile_pool(name="psp", bufs=2, space="PSUM")
                    for tt in range(12):
                        for oc in range(2):
                            thunks.append(
                                lambda tt=tt, oc=oc: proj_tile(tt, oc, False, psp))
                attn(qc, thunks)

            # ---------------- projection tail (q-chunk 3) ----------------
            for tt in range(12, 16):
                for oc in range(2):
                    proj_tile(tt, oc, (tt + oc) % 2 == 1, psp)

            obp2.release()
            xs.release()
            p1.release()
            rp.release()
            es.release()
            psp.release()
            pso.release()
            pss.release()

    nc.compile()
    return nc


def _in_maps(x, Wk, bk, Wq, bq, Wv, bv, Wp, bp, padding_mask):
    maps = []
    mask_cols = np.arange(896)[None, :]
    mask_rows = np.arange(128)[:, None]
    maskneg = np.where(mask_rows <= mask_cols - 384, 0.0, NEG).astype(np.float32)
    identb = np.eye(128).astype(ml_dtypes.bfloat16)
    maskb = maskneg[:, 384:512].astype(ml_dtypes.bfloat16)
    constb = np.concatenate([identb, maskb], axis=1)
    for core in range(8):
        b, half = divmod(core, 2)
        hs = slice(half * IC, (half + 1) * IC)
        constf = np.concatenate([
            np.ascontiguousarray((bq[hs] * SCALE).reshape(4, 128).T),
            np.ascontiguousarray(bk[hs].reshape(4, 128).T),
            np.ascontiguousarray(
                np.where(padding_mask[b] != 0, 0.0, NEG)
                .astype(np.float32).reshape(NKT, 128).T),
        ], axis=1).astype(np.float32)
        maps.append({
            "xT": np.ascontiguousarray(x[b].T),
            "WqT": np.ascontiguousarray(Wq[hs, :].T),
            "WkT": np.ascontiguousarray(Wk[hs, :].T),
            "WvT": np.ascontiguousarray(Wv[hs, :].T),
            "WpT": np.ascontiguousarray(Wp[:, hs].T),
            "constf": np.ascontiguousarray(constf),
            "constb": np.ascontiguousarray(constb),
            "bvr": bv[hs].reshape(1, IC).copy(),
            "ones1": np.ones((1, 128), np.float32),
            "ones8b": np.ones((128, 8), ml_dtypes.bfloat16),
        })
    return maps


def _run(inputs, trace=False, **kw):
    if "nc" not in _CACHE:
        _CACHE["nc"] = _build()
    nc = _CACHE["nc"]
    ins = {k: np.asarray(v, dtype=np.float32) if k != "padding_mask"
           else np.asarray(v) for k, v in inputs.items()}
    maps = _in_maps(**ins)
    res = run_bass_kernel_spmd(nc, maps, core_ids=list(range(8)), trace=trace, **kw)
    bp = np.asarray(inputs["bp"], np.float32)
    y = np.empty((B, T, C), np.float32)
    for b in range(B):
        y[b] = res.results[2 * b]["out"] + res.results[2 * b + 1]["out"] + bp
    return y, res


def kernel(**inputs):
    y, _ = _run(inputs, trace=False)
    return y


# revision 11
# speedup vs baseline: 1.2733x; 1.0208x over previous
"""Causal self-attention TRN2 Bass kernel — iteration 2.

Problem: B=4, T=2048, C=1024, H=16 heads (HD=64), torch-Linear semantics
(y = x @ W.T + b), causal + padding mask, softmax, output projection.

Sharding: 8 cores = (batch b in 0..3) x (head-half in 0..1). Each core
handles one batch and 8 heads; the two half-cores of a batch produce
partial output projections that the host sums (plus bp).

Key scheduling idea vs iteration 1: the attention softmax (exp on the
ACT engine, ~1us per 128-k-tile) is the second-largest engine load after
the PE matmuls. Instead of running QKV projection, attention, and output
projection as serial phases (which leaves ACT idle during QKV/proj and
makes attention an ACT/PE lockstep), attention for q-chunk qc is
interleaved with the QKV projection chains of chunk qc+1 (attention only
needs K/V chunks <= qc), and the output projection of q-chunks 0..2 is
interleaved into attention of qc=3. The PE instruction stream then always
has dense matmul work while exp runs concurrently, and only qc=3's
projection remains as a short tail.

Numerics: QKV/proj matmuls in float32r; Q/K/V and softmax probs in bf16
(attention matmuls all-bf16; ~1.5e-3 worst-case rel err vs the 2e-2
harness gate). Normalization uses the exact DVE reciprocal + gpsimd row
broadcast (approx-recip / ACT ln-exp variants miscompute on this HW).
"""

from collections import deque

import ml_dtypes
import numpy as np

import concourse.mybir as mybir
import concourse.tile as tile
from concourse import bacc
from concourse.bass_utils import run_bass_kernel_spmd

F32 = mybir.dt.float32
F32R = mybir.dt.float32r
BF16 = mybir.dt.bfloat16
AF = mybir.ActivationFunctionType
ALU = mybir.AluOpType

B, T, C, H = 4, 2048, 1024, 16
HD = C // H          # 64
IC = C // 2          # 512 channels per core (8 heads)
NKT = T // 128       # 16 k-tiles
NCT = C // 128       # 8 contraction tiles for QKV
NEG = -1.0e30
SCALE = 1.0 / np.sqrt(HD)
D = 5                # flash pipeline depth (k-tiles between S and O)

_CACHE = {}


def _build():
    nc = bacc.Bacc("TRN2", target_bir_lowering=False, debug=False)

    xT_d = nc.dram_tensor("xT", [C, T], F32, kind="ExternalInput").ap()
    WqT_d = nc.dram_tensor("WqT", [C, IC], F32, kind="ExternalInput").ap()
    WkT_d = nc.dram_tensor("WkT", [C, IC], F32, kind="ExternalInput").ap()
    WvT_d = nc.dram_tensor("WvT", [C, IC], F32, kind="ExternalInput").ap()
    WpT_d = nc.dram_tensor("WpT", [IC, C], BF16, kind="ExternalInput").ap()
    # packed constants: cols 0-3 bq*SCALE, 4-7 bk, 8-23 padding bias
    constf_d = nc.dram_tensor("constf", [128, 24], F32, kind="ExternalInput").ap()
    # packed bf16 constants: cols 0-127 identity, 128-255 causal mask bias
    constb_d = nc.dram_tensor("constb", [128, 256], BF16, kind="ExternalInput").ap()
    bv_d = nc.dram_tensor("bvr", [1, IC], F32, kind="ExternalInput").ap()
    ones1_d = nc.dram_tensor("ones1", [1, 128], F32, kind="ExternalInput").ap()
    ones8_d = nc.dram_tensor("ones8b", [128, 8], BF16, kind="ExternalInput").ap()
    out_d = nc.dram_tensor("out", [T, C], F32, kind="ExternalOutput").ap()

    with tile.TileContext(nc) as tc:
        with tc.tile_pool(name="pp", bufs=1) as pp:
            # Persistent SBUF state
            QT = pp.tile([128, 4 * T], BF16, name="QT")     # 4 head-pair tiles
            KT = pp.tile([128, 4 * T], BF16, name="KT")
            Vt = pp.tile([128, NKT * 520], BF16, name="Vt")  # [V|1] x 8 heads
            YT = pp.tile([128, 4 * T], BF16, name="YT")
            Wp_sb = pp.tile([128, 4 * C], BF16, name="Wp_sb")
            constf = pp.tile([128, 24], F32, name="constf")
            constb = pp.tile([128, 256], BF16, name="constb")
            bv_sb = pp.tile([1, IC], F32R, name="bv_sb")
            ones128 = pp.tile([1, 128], F32R, name="ones128")
            bq_sb = constf[:, 0:4]
            bk_sb = constf[:, 4:8]
            pad_sb = constf[:, 8:24]
            ident_sb = constb[:, 0:128]
            mask_sb = constb[:, 128:256]
            Vr = Vt.rearrange("p (k h c) -> p k h c", k=NKT, h=8, c=65)

            # PSUM pools stack: pss, pso stay for the whole kernel; ps1 (on
            # top) is swapped for the projection pool psp once QKV is done.
            pss = tc.alloc_tile_pool(name="pss", bufs=2, space="PSUM")
            pso = tc.alloc_tile_pool(name="pso", bufs=1, space="PSUM")
            ps1 = tc.alloc_tile_pool(name="ps1", bufs=2, space="PSUM")
            es = tc.alloc_tile_pool(name="es", bufs=D + 1)
            rp = tc.alloc_tile_pool(name="rp", bufs=1)
            p1 = tc.alloc_tile_pool(name="p1", bufs=1)
            xs = tc.alloc_tile_pool(name="xs", bufs=3)
            obp2 = tc.alloc_tile_pool(name="ob2", bufs=2)
            Wq_l = [p1.tile([128, 512], F32R, name=f"Wq{ct}") for ct in range(NCT)]
            Wk_l = [p1.tile([128, 512], F32R, name=f"Wk{ct}") for ct in range(NCT)]
            Wv_l = [p1.tile([128, 512], F32R, name=f"Wv{ct}") for ct in range(NCT)]

            def dma_x(ch):
                t0 = ch * 512
                xc = [xs.tile([128, 512], F32R, name=f"xc{ct}", tag=f"xc{ct}")
                      for ct in range(NCT)]
                for ct in range(NCT):
                    nc.sync.dma_start(
                        out=xc[ct][:],
                        in_=xT_d[ct * 128:(ct + 1) * 128, t0:t0 + 512].bitcast(F32R))
                return xc

            def q_chain(ch, g, xc):
                t0 = ch * 512
                pq = ps1.tile([128, 512], F32, name="pq", tag="p1ps")
                for ct in range(NCT):
                    nc.tensor.matmul(
                        out=pq[:], lhsT=Wq_l[ct][:, g * 128:(g + 1) * 128],
                        rhs=xc[ct][:], start=(ct == 0), stop=(ct == NCT - 1))
                nc.scalar.activation(
                    QT[:, g * T + t0: g * T + t0 + 512], pq[:], AF.Identity,
                    scale=SCALE, bias=bq_sb[:, g:g + 1])

            def k_chain(ch, g, xc):
                t0 = ch * 512
                pk = ps1.tile([128, 512], F32, name="pk", tag="p1ps")
                for ct in range(NCT):
                    nc.tensor.matmul(
                        out=pk[:], lhsT=Wk_l[ct][:, g * 128:(g + 1) * 128],
                        rhs=xc[ct][:], start=(ct == 0), stop=(ct == NCT - 1))
                nc.scalar.activation(
                    KT[:, g * T + t0: g * T + t0 + 512], pk[:], AF.Identity,
                    bias=bk_sb[:, g:g + 1])

            def v_chain(ch, ts, xc):
                kt = ch * 4 + ts
                pv_ = ps1.tile([128, 512], F32, name="pv_", tag="p1ps")
                for ct in range(NCT):
                    nc.tensor.matmul(
                        out=pv_[:], lhsT=xc[ct][:, ts * 128: ts * 128 + 128],
                        rhs=Wv_l[ct][:], start=(ct == 0), stop=False)
                nc.tensor.matmul(
                    out=pv_[:], lhsT=ones128[:], rhs=bv_sb[:],
                    start=False, stop=True)
                nc.scalar.copy(Vr[:, kt, :, 0:64], pv_[:])

            def proj_tile(tt, oc, on_act, psp):
                po = psp.tile([128, 512], F32, name="po", tag="po")
                for g2 in range(4):
                    nc.tensor.matmul(
                        out=po[:],
                        lhsT=YT[:, g2 * T + tt * 128: g2 * T + tt * 128 + 128],
                        rhs=Wp_sb[:, g2 * C + oc * 512: g2 * C + oc * 512 + 512],
                        start=(g2 == 0), stop=(g2 == 3))
                ob = obp2.tile([128, 512], F32, name="ob", tag="ob")
                if on_act:
                    nc.scalar.copy(ob[:], po[:])
                else:
                    nc.vector.tensor_copy(ob[:], po[:])
                nc.sync.dma_start(
                    out=out_d[tt * 128:(tt + 1) * 128, oc * 512:(oc + 1) * 512],
                    in_=ob[:])

            def attn(qc, thunks):
                q0 = qc * 512
                kmax = 4 * qc + 4
                total_steps = 4 * (kmax + D)
                spacing = max(1, total_steps // max(1, len(thunks)))
                sidx = 0
                for g in range(4):
                    gq = g * T
                    oAB = pso.tile([65, 1024], F32, name="oAB", tag="o")
                    e_l = [None] * kmax
                    off_l = [None] * kmax
                    for step in range(kmax + D):
                        if step < kmax:
                            kt = step
                            k0 = kt * 128
                            toff = 128 * (kt - 4 * qc) if kt >= 4 * qc else 0
                            off_l[kt] = toff
                            diag = kt >= 4 * qc
                            sAB = pss.tile([128, 1024], F32, name="sAB", tag="sAB")
                            nc.tensor.matmul(
                                out=sAB[:, toff:512],
                                lhsT=KT[0:64, gq + k0: gq + k0 + 128],
                                rhs=QT[0:64, gq + q0 + toff: gq + q0 + 512],
                                start=True, stop=not diag)
                            nc.tensor.matmul(
                                out=sAB[:, 512 + toff:1024],
                                lhsT=KT[64:128, gq + k0: gq + k0 + 128],
                                rhs=QT[64:128, gq + q0 + toff: gq + q0 + 512],
                                start=True, stop=not diag, tile_position=(64, 0))
                            if diag:
                                # additive causal mask on the 128-wide
                                # diagonal band, via identity matmul
                                nc.tensor.matmul(
                                    out=sAB[:, toff:toff + 128],
                                    lhsT=ident_sb, rhs=mask_sb,
                                    start=False, stop=True)
                                nc.tensor.matmul(
                                    out=sAB[:, 512 + toff:512 + toff + 128],
                                    lhsT=ident_sb, rhs=mask_sb,
                                    start=False, stop=True)
                            eAB = es.tile([128, 1024], BF16, name="eAB", tag="eAB")
                            s3 = sAB.rearrange("p (h w) -> p h w", h=2, w=512)
                            e3 = eAB.rearrange("p (h w) -> p h w", h=2, w=512)
                            nc.scalar.activation(
                                e3[:, :, toff:512], s3[:, :, toff:512], AF.Exp,
                                bias=pad_sb[:, kt:kt + 1])
                            e_l[kt] = eAB
                        pv = step - D
                        if 0 <= pv < kmax:
                            toff = off_l[pv]
                            vbase = pv * 520
                            nc.tensor.matmul(
                                out=oAB[:, toff:512],
                                lhsT=Vt[:, vbase + 130 * g: vbase + 130 * g + 65],
                                rhs=e_l[pv][:, toff:512],
                                start=(pv == 0), stop=(pv == kmax - 1))
                            nc.tensor.matmul(
                                out=oAB[:, 512 + toff:1024],
                                lhsT=Vt[:, vbase + 130 * g + 65: vbase + 130 * g + 130],
                                rhs=e_l[pv][:, 512 + toff:1024],
                                start=(pv == 0), stop=(pv == kmax - 1))
                        sidx += 1
                        if thunks and sidx % spacing == 0:
                            thunks.popleft()()
                    # epilogue: normalize by rowsum (row 64), write Y^T.
                    # Exact DVE reciprocal (slow, ~6 cyc/elem, but the only
                    # normalization primitive that is bit-correct on this
                    # hardware runtime); the interleaved thunks keep the PE
                    # fed while this chain runs.
                    ocp = rp.tile([65, 1024], F32, name="ocp", tag="ocp")
                    nc.scalar.copy(ocp[:], oAB[:])
                    rA = rp.tile([1, 512], F32, name="rA", tag="rA")
                    rB = rp.tile([1, 512], F32, name="rB", tag="rB")
                    nc.vector.reciprocal(rA[:], ocp[64:65, 0:512])
                    nc.vector.reciprocal(rB[:], ocp[64:65, 512:1024])
                    rbA = rp.tile([64, 512], F32, name="rbA", tag="rbA")
                    rbB = rp.tile([64, 512], F32, name="rbB", tag="rbB")
                    nc.gpsimd.partition_broadcast(rbA[:], rA[:])
                    nc.gpsimd.partition_broadcast(rbB[:], rB[:])
                    nc.vector.tensor_mul(
                        YT[0:64, gq + q0: gq + q0 + 512],
                        ocp[0:64, 0:512], rbA[:])
                    nc.vector.tensor_mul(
                        YT[64:128, gq + q0: gq + q0 + 512],
                        ocp[0:64, 512:1024], rbB[:])
                while thunks:
                    thunks.popleft()()

            # ---------------- emission ----------------
            # chunk 0: critical-path DMA order, then QKV(0) inline
            xc0 = [xs.tile([128, 512], F32R, name=f"xc{ct}", tag=f"xc{ct}")
                   for ct in range(NCT)]
            for ct in range(NCT):
                cs = slice(ct * 128, (ct + 1) * 128)
                nc.sync.dma_start(out=xc0[ct][:], in_=xT_d[cs, 0:512].bitcast(F32R))
                nc.sync.dma_start(out=Wq_l[ct][:], in_=WqT_d[cs, :].bitcast(F32R))
            nc.sync.dma_start(out=constf[:], in_=constf_d)
            nc.sync.dma_start(out=constb[:], in_=constb_d)
            nc.sync.dma_start(out=bv_sb[:], in_=bv_d.bitcast(F32R))
            nc.sync.dma_start(out=ones128[:], in_=ones1_d.bitcast(F32R))
            for ct in range(NCT):
                cs = slice(ct * 128, (ct + 1) * 128)
                nc.sync.dma_start(out=Wk_l[ct][:], in_=WkT_d[cs, :].bitcast(F32R))
            for kt in range(NKT):
                nc.sync.dma_start(out=Vr[:, kt, :, 64], in_=ones8_d)
            for ct in range(NCT):
                cs = slice(ct * 128, (ct + 1) * 128)
                nc.sync.dma_start(out=Wv_l[ct][:], in_=WvT_d[cs, :].bitcast(F32R))
            for g in range(4):
                nc.sync.dma_start(
                    out=Wp_sb[:, g * C:(g + 1) * C],
                    in_=WpT_d[g * 128:(g + 1) * 128, :])
            for g in range(4):
                q_chain(0, g, xc0)
            for g in range(4):
                k_chain(0, g, xc0)
            for ts in range(4):
                v_chain(0, ts, xc0)

            # attention(qc) interleaved with QKV(qc+1) chains. x chunks
            # are prefetched TWO regions ahead (xs bufs=3) so the thunk
            # chains never head-block the PE FIFO on an in-flight DMA.
            psp = None
            xc_pref = {1: dma_x(1), 2: dma_x(2)}
            for qc in range(4):
                thunks = deque()
                if qc < 3:
                    xc = xc_pref.pop(qc + 1)
                    if qc + 3 <= 3:
                        xc_pref[qc + 3] = dma_x(qc + 3)
                    for g in range(4):
                        thunks.append(lambda g=g, xc=xc: q_chain(qc + 1, g, xc))
                    for g in range(4):
                        thunks.append(lambda g=g, xc=xc: k_chain(qc + 1, g, xc))
                    for ts in range(4):
                        thunks.append(lambda ts=ts, xc=xc: v_chain(qc + 1, ts, xc))
                else:
                    # last QKV done: swap ps1's banks for the projection pool
                    # and interleave the projection of q-chunks 0..2
                    ps1.release()
                    psp = tc.alloc_tile_pool(name="psp", bufs=2, space="PSUM")
                    for tt in range(12):
                        for oc in range(2):
                            thunks.append(
                                lambda tt=tt, oc=oc: proj_tile(tt, oc, False, psp))
                attn(qc, thunks)

            # ---------------- projection tail (q-chunk 3) ----------------
            for tt in range(12, 16):
                for oc in range(2):
                    proj_tile(tt, oc, (tt + oc) % 2 == 1, psp)

            obp2.release()
            xs.release()
            p1.release()
            rp.release()
            es.release()
            psp.release()
            pso.release()
            pss.release()

    nc.compile()
    return nc


def _in_maps(x, Wk, bk, Wq, bq, Wv, bv, Wp, bp, padding_mask):
    maps = []
    mask_cols = np.arange(896)[None, :]
    mask_rows = np.arange(128)[:, None]
    maskneg = np.where(mask_rows <= mask_cols - 384, 0.0, NEG).astype(np.float32)
    identb = np.eye(128).astype(ml_dtypes.bfloat16)
    maskb = maskneg[:, 384:512].astype(ml_dtypes.bfloat16)
    constb = np.concatenate([identb, maskb], axis=1)
    for core in range(8):
        b, half = divmod(core, 2)
        hs = slice(half * IC, (half + 1) * IC)
        constf = np.concatenate([
            np.ascontiguousarray((bq[hs] * SCALE).reshape(4, 128).T),
            np.ascontiguousarray(bk[hs].reshape(4, 128).T),
            np.ascontiguousarray(
                np.where(padding_mask[b] != 0, 0.0, NEG)
                .astype(np.float32).reshape(NKT, 128).T),
        ], axis=1).astype(np.float32)
        maps.append({
            "xT": np.ascontiguousarray(x[b].T),
            "WqT": np.ascontiguousarray(Wq[hs, :].T),
            "WkT": np.ascontiguousarray(Wk[hs, :].T),
            "WvT": np.ascontiguousarray(Wv[hs, :].T),
            "WpT": np.ascontiguousarray(Wp[:, hs].T.astype(ml_dtypes.bfloat16)),
            "constf": np.ascontiguousarray(constf),
            "constb": np.ascontiguousarray(constb),
            "bvr": bv[hs].reshape(1, IC).copy(),
            "ones1": np.ones((1, 128), np.float32),
            "ones8b": np.ones((128, 8), ml_dtypes.bfloat16),
        })
    return maps


def _run(inputs, trace=False, **kw):
    if "nc" not in _CACHE:
        _CACHE["nc"] = _build()
    nc = _CACHE["nc"]
    ins = {k: np.asarray(v, dtype=np.float32) if k != "padding_mask"
           else np.asarray(v) for k, v in inputs.items()}
    maps = _in_maps(**ins)
    res = run_bass_kernel_spmd(nc, maps, core_ids=list(range(8)), trace=trace, **kw)
    bp = np.asarray(inputs["bp"], np.float32)
    y = np.empty((B, T, C), np.float32)
    for b in range(B):
        y[b] = res.results[2 * b]["out"] + res.results[2 * b + 1]["out"] + bp
    return y, res


def kernel(**inputs):
    y, _ = _run(inputs, trace=False)
    return y
